# revision 23
# baseline (speedup 1.0000x reference)
"""Trainium2 Bass kernel for: flatten -> dense(relu) -> freq-count -> dense -> softmax.

reference:
    flat = x.reshape(B, 784)
    h    = relu(flat @ W1 + b1)                  # [B, 100]
    freq = freq + sum((h[:-1] > 0), axis=0)      # [100]
    out  = softmax(h @ W2 + b2, axis=-1)         # [B, 10]

Strategy: pure data-parallel over 8 NeuronCores (batch sharding).  The host
pre-transposes each x shard to feature-major [784, 8192] (pure layout change,
same bytes streamed from HBM) so the contraction dim lands on SBUF partitions
without any on-device transposition.  Per-shard freq increments are summed on
the host (the "all-reduce"); the reference's exclusion of the last batch row
is handled by subtracting core 7's last-row indicator, which the device
computes from the same PSUM values used for the counts.
"""

from contextlib import ExitStack

import numpy as np

import concourse.bass as bass
import concourse.tile as tile
from concourse import bacc, mybir
from concourse.bass_utils import run_bass_kernel_spmd

F32 = mybir.dt.float32
F32R = mybir.dt.float32r
F16 = mybir.dt.float16

N_CORES = 8
B = 65536
D_IN = 784
D_HID = 100
D_OUT = 10
B_SHARD = B // N_CORES          # 8192 rows per core
TILE = 512                      # batch rows per pipeline tile
N_TILES = B_SHARD // TILE       # 16
KP = 112                        # contraction-chunk partitions (7 * 112 = 784)
KC = D_IN // KP                 # 7 accumulating matmuls per tile
J = TILE // 128                 # 4 output sub-blocks of 128 rows
DH1 = D_HID + 1                 # h augmented with a ones-row => free b2 add
ST = 2                          # tiles per DMA stage (fp16: 14KB/partition runs)
N_STAGES = N_TILES // ST


def _build_body(ctx: ExitStack, tc: tile.TileContext, aps: dict):
    nc = tc.nc
    xb, w1x, wc = aps["xb"], aps["W1X"], aps["wc"]
    out, freqc, indlast = aps["out"], aps["freqc"], aps["indlast"]

    singles = ctx.enter_context(tc.tile_pool(name="singles", bufs=1))
    xp = ctx.enter_context(tc.tile_pool(name="xp", bufs=6))
    hp = ctx.enter_context(tc.tile_pool(name="hp", bufs=3))
    indp = ctx.enter_context(tc.tile_pool(name="indp", bufs=2))
    smp = ctx.enter_context(tc.tile_pool(name="smp", bufs=3))
    ps_h = ctx.enter_context(tc.tile_pool(name="ps_h", bufs=2, space="PSUM"))
    ps_l = ctx.enter_context(tc.tile_pool(name="ps_l", bufs=2, space="PSUM"))

    # --- one-time constants, two DMAs total ---------------------------------
    # W1X: fp16 [112, 7*101 + 10]: 7 chunks of W1 (zero-padded col 100) plus
    # W2b (W2 with b2 as row 100) in the last 10 columns.
    # wc:  f32 [128, 2]: col 0 = b1 (with 1.0 at row 100), col 1 = -b1.
    w1x_sb = singles.tile([KP, KC * DH1 + D_OUT], F16)
    nc.sync.dma_start(out=w1x_sb[:, :], in_=w1x)
    wc_sb = singles.tile([128, 2], F32)
    nc.sync.dma_start(out=wc_sb[:, :], in_=wc)

    b1_ap = wc_sb[0:DH1, 0:1]
    negb1 = wc_sb[0:D_HID, 1:2]
    w2b_ap = w1x_sb[0:DH1, KC * DH1:KC * DH1 + D_OUT]

    freq_sb = singles.tile([D_HID, N_TILES], F32)
    il_sb = singles.tile([D_HID, 1], F32)
    out_all = singles.tile([128, N_TILES, J, D_OUT], F32)

    def softmax_emit(t, h_sb):
        # logits + softmax; all 4 j-blocks share one PSUM tile
        lps = ps_l.tile([128, J, D_OUT], F32, tag="lps")
        for j in range(J):
            nc.tensor.matmul(
                lps[:, j, :],
                lhsT=h_sb[:, bass.ts(j, 128)],
                rhs=w2b_ap,
                start=True,
                stop=True,
            )
        exp_sb = smp.tile([128, J, D_OUT], F32, tag="exp")
        nc.scalar.activation(out=exp_sb[:, :, :], in_=lps[:, :, :],
                             func=mybir.ActivationFunctionType.Exp)
        den = smp.tile([128, J], F32, tag="den")
        nc.vector.reduce_sum(out=den[:, :], in_=exp_sb[:, :, :],
                             axis=mybir.AxisListType.X)
        rec = smp.tile([128, J], F32, tag="rec")
        nc.vector.reciprocal(rec[:, :], den[:, :])
        nc.vector.tensor_tensor(
            out=out_all[:, t, :, :], in0=exp_sb[:, :, :],
            in1=rec.unsqueeze(2).broadcast_to([128, J, D_OUT]),
            op=mybir.AluOpType.mult)

    # --- main pipeline (softmax of tile t-1 emitted after matmuls of t) -----
    pending = None
    for s in range(N_STAGES):
        x_sb = xp.tile([KP, ST, KC, TILE], F16)
        eng = nc.sync if s % 2 == 0 else nc.scalar
        eng.dma_start(out=x_sb[:, :, :, :], in_=xb[s])

        for sub in range(ST):
            t = s * ST + sub

            hps = ps_h.tile([DH1, TILE], F32)
            for c in range(KC):
                nc.tensor.matmul(
                    hps[:, :],
                    lhsT=w1x_sb[:, c * DH1:(c + 1) * DH1],
                    rhs=x_sb[:, sub, c, :],
                    start=(c == 0),
                    stop=(c == KC - 1),
                )

            # h = relu(x@W1 + b1), with the ones-row at partition 100
            h_sb = hp.tile([DH1, TILE], F16)
            nc.scalar.activation(out=h_sb[:, :], in_=hps[:, :],
                                 func=mybir.ActivationFunctionType.Relu,
                                 bias=b1_ap)

            # freq count: (h > 0) summed along the batch (free) dim.
            # h is post-relu fp16; its rounding flips the f32 sign only for
            # |h| < 2^-24, so the count matches the f32 indicator.
            ind_sb = indp.tile([D_HID, TILE], F16)
            nc.vector.tensor_scalar(
                out=ind_sb[:, :],
                in0=h_sb[0:D_HID, :],
                scalar1=0.0,
                scalar2=None,
                op0=mybir.AluOpType.is_gt,
                op1=mybir.AluOpType.add,
                accum_out=freq_sb[:, t:t + 1],
            )
            if t == N_TILES - 1:
                # indicator of the shard's last row (global last on core 7)
                nc.vector.tensor_scalar(
                    out=il_sb[:, :],
                    in0=h_sb[0:D_HID, TILE - 1:TILE],
                    scalar1=0.0,
                    scalar2=None,
                    op0=mybir.AluOpType.is_gt,
                )

            if pending is not None:
                tprev = pending[0]
                softmax_emit(*pending)
                if tprev == N_TILES // 2 - 1:
                    half = N_TILES // 2
                    nc.sync.dma_start(out=out[:, 0:half],
                                      in_=out_all[:, 0:half, :, :])
            pending = (t, h_sb)

    softmax_emit(*pending)

    half = N_TILES // 2
    nc.sync.dma_start(out=out[:, half:], in_=out_all[:, half:, :, :])
    nc.sync.dma_start(out=freqc, in_=freq_sb[:, :])
    nc.sync.dma_start(out=indlast, in_=il_sb[:, :])


def build_nc():
    nc = bacc.Bacc("TRN2", target_bir_lowering=False, debug=False,
                   num_devices=N_CORES)
    aps = {
        "xb": nc.dram_tensor("xb", [N_STAGES, KP, ST, KC, TILE], F16,
                             kind="ExternalInput").ap(),
        "W1X": nc.dram_tensor("W1X", [KP, KC * DH1 + D_OUT], F16,
                              kind="ExternalInput").ap(),
        "wc": nc.dram_tensor("wc", [128, 2], F32, kind="ExternalInput").ap(),
        # out is stored blocked [p, t, j, k]; host restores batch order
        "out": nc.dram_tensor("out", [128, N_TILES, J, D_OUT], F32,
                              kind="ExternalOutput").ap(),
        "freqc": nc.dram_tensor("freqc", [D_HID, N_TILES], F32,
                                kind="ExternalOutput").ap(),
        "indlast": nc.dram_tensor("indlast", [D_HID, 1], F32,
                                  kind="ExternalOutput").ap(),
    }
    with tile.TileContext(nc) as tc, ExitStack() as ctx:
        _build_body(ctx, tc, aps)
    nc.compile()
    return nc


_NC = None


def _get_nc():
    global _NC
    if _NC is None:
        _NC = build_nc()
    return _NC


def make_in_maps(x, W1, b1, W2, b2):
    xf = np.ascontiguousarray(np.asarray(x, dtype=np.float32).reshape(B, D_IN))
    W1 = np.asarray(W1, dtype=np.float32)
    b1 = np.asarray(b1, dtype=np.float32)
    W2 = np.asarray(W2, dtype=np.float32)
    b2 = np.asarray(b2, dtype=np.float32)
    # host-side weight augmentation + blocking (layout only; see _build_body)
    W1p = np.concatenate([W1, np.zeros((D_IN, 1), np.float32)], axis=1)
    W1X = np.zeros((KP, KC * DH1 + D_OUT), np.float16)
    W1X[:, :KC * DH1] = (W1p.reshape(KC, KP, DH1).transpose(1, 0, 2)
                         .reshape(KP, KC * DH1).astype(np.float16))
    W2b = np.concatenate([W2, b2[None, :]], axis=0)           # [101, 10]
    W1X[0:DH1, KC * DH1:] = W2b.astype(np.float16)
    wc = np.zeros((128, 2), np.float32)
    wc[0:D_HID, 0] = b1
    wc[D_HID, 0] = 1.0
    wc[0:D_HID, 1] = -b1
    in_maps = []
    for c in range(N_CORES):
        shard = xf[c * B_SHARD:(c + 1) * B_SHARD]             # [8192, 784]
        # xb[s, p, sub, ch, col] = shard[(s*ST+sub)*TILE + col, ch*KP + p]
        xbt = shard.reshape(N_STAGES, ST, TILE, KC, KP).transpose(0, 4, 1, 3, 2)
        in_maps.append({
            "xb": np.ascontiguousarray(xbt.astype(np.float16)),
            "W1X": W1X, "wc": wc,
        })
    return in_maps


def postprocess(results, freq):
    outs = []
    for c in range(N_CORES):
        o = results[c]["out"]                             # [128, 16, 4, 10]
        outs.append(o.transpose(1, 2, 0, 3).reshape(B_SHARD, D_OUT))
    out = np.concatenate(outs, axis=0)                    # [65536, 10]
    freq_new = np.asarray(freq, dtype=np.float32).copy()
    for c in range(N_CORES):
        freq_new += results[c]["freqc"].sum(axis=1)
    freq_new -= results[N_CORES - 1]["indlast"][:, 0]
    return out, freq_new.astype(np.float32)


def kernel(x, W1, b1, freq, W2, b2):
    nc = _get_nc()
    in_maps = make_in_maps(x, W1, b1, W2, b2)
    res = run_bass_kernel_spmd(nc, in_maps, list(range(N_CORES)))
    return postprocess(res.results, freq)


# revision 24
# speedup vs baseline: 1.1856x; 1.1856x over previous
"""Trainium2 Bass kernel for: flatten -> dense(relu) -> freq-count -> dense -> softmax.

reference:
    flat = x.reshape(B, 784)
    h    = relu(flat @ W1 + b1)                  # [B, 100]
    freq = freq + sum((h[:-1] > 0), axis=0)      # [100]
    out  = softmax(h @ W2 + b2, axis=-1)         # [B, 10]

Strategy: pure data-parallel over 8 NeuronCores (batch sharding).  The host
pre-transposes each x shard to feature-major [784, 8192] (pure layout change,
same bytes streamed from HBM) so the contraction dim lands on SBUF partitions
without any on-device transposition.  Per-shard freq increments are summed on
the host (the "all-reduce"); the reference's exclusion of the last batch row
is handled by subtracting core 7's last-row indicator, which the device
computes from the same PSUM values used for the counts.
"""

from contextlib import ExitStack

import numpy as np

import concourse.bass as bass
import concourse.tile as tile
from concourse import bacc, mybir
from concourse.bass_utils import run_bass_kernel_spmd

F32 = mybir.dt.float32
F32R = mybir.dt.float32r
F16 = mybir.dt.float16

N_CORES = 8
B = 65536
D_IN = 784
D_HID = 100
D_OUT = 10
B_SHARD = B // N_CORES          # 8192 rows per core
TILE = 512                      # batch rows per pipeline tile
N_TILES = B_SHARD // TILE       # 16
KP = 112                        # contraction-chunk partitions (7 * 112 = 784)
KC = D_IN // KP                 # 7 accumulating matmuls per tile
J = TILE // 128                 # 4 output sub-blocks of 128 rows
DH1 = D_HID + 1                 # h augmented with a ones-row => free b2 add
ST = 2                          # tiles per DMA stage (fp16: 14KB/partition runs)
N_STAGES = N_TILES // ST


def _build_body(ctx: ExitStack, tc: tile.TileContext, aps: dict):
    nc = tc.nc
    xb, w1x, wc = aps["xb"], aps["W1X"], aps["wc"]
    out, freqc = aps["out"], aps["freqc"]

    singles = ctx.enter_context(tc.tile_pool(name="singles", bufs=1))
    xp = ctx.enter_context(tc.tile_pool(name="xp", bufs=6))
    hp = ctx.enter_context(tc.tile_pool(name="hp", bufs=3))
    indp = ctx.enter_context(tc.tile_pool(name="indp", bufs=2))
    smp = ctx.enter_context(tc.tile_pool(name="smp", bufs=3))
    ps_h = ctx.enter_context(tc.tile_pool(name="ps_h", bufs=2, space="PSUM"))
    ps_l = ctx.enter_context(tc.tile_pool(name="ps_l", bufs=2, space="PSUM"))

    # --- first x stage goes first so its descriptors lead the ring ----------
    x_first = xp.tile([KP, ST, KC, TILE], F16, tag="x_sb")
    nc.sync.dma_start(out=x_first[:, :, :, :], in_=xb[0])

    # --- one-time constants, two DMAs total ---------------------------------
    # W1X: fp16 [112, 7*101 + 10]: 7 chunks of W1 (zero-padded col 100) plus
    # W2b (W2 with b2 as row 100) in the last 10 columns.
    # wc:  f32 [128, 2]: col 0 = b1 (with 1.0 at row 100), col 1 = -b1.
    w1x_sb = singles.tile([KP, KC * DH1 + D_OUT], F16)
    nc.sync.dma_start(out=w1x_sb[:, :], in_=w1x)
    wc_sb = singles.tile([128, 2], F32)
    nc.sync.dma_start(out=wc_sb[:, :], in_=wc)

    b1_ap = wc_sb[0:DH1, 0:1]
    negb1 = wc_sb[0:D_HID, 1:2]
    w2b_ap = w1x_sb[0:DH1, KC * DH1:KC * DH1 + D_OUT]

    freq_sb = singles.tile([D_HID, N_TILES + 1], F32)
    il_sb = freq_sb[:, N_TILES:N_TILES + 1]
    out_all = singles.tile([128, N_TILES, J, D_OUT], F32)

    def softmax_emit(t, h_sb):
        # logits + softmax; all 4 j-blocks share one PSUM tile
        lps = ps_l.tile([128, J, D_OUT], F32, tag="lps")
        for j in range(J):
            nc.tensor.matmul(
                lps[:, j, :],
                lhsT=h_sb[:, bass.ts(j, 128)],
                rhs=w2b_ap,
                start=True,
                stop=True,
            )
        exp_sb = smp.tile([128, J, D_OUT], F32, tag="exp")
        nc.scalar.activation(out=exp_sb[:, :, :], in_=lps[:, :, :],
                             func=mybir.ActivationFunctionType.Exp)
        den = smp.tile([128, J], F32, tag="den")
        nc.vector.reduce_sum(out=den[:, :], in_=exp_sb[:, :, :],
                             axis=mybir.AxisListType.X)
        rec = smp.tile([128, J], F32, tag="rec")
        nc.vector.reciprocal(rec[:, :], den[:, :])
        nc.vector.tensor_tensor(
            out=out_all[:, t, :, :], in0=exp_sb[:, :, :],
            in1=rec.unsqueeze(2).broadcast_to([128, J, D_OUT]),
            op=mybir.AluOpType.mult)

    # --- main pipeline (softmax of tile t-1 emitted after matmuls of t) -----
    pending = None
    for s in range(N_STAGES):
        if s == 0:
            x_sb = x_first
        else:
            x_sb = xp.tile([KP, ST, KC, TILE], F16, tag="x_sb")
            nc.sync.dma_start(out=x_sb[:, :, :, :], in_=xb[s])

        for sub in range(ST):
            t = s * ST + sub

            hps = ps_h.tile([DH1, TILE], F32)
            for c in range(KC):
                nc.tensor.matmul(
                    hps[:, :],
                    lhsT=w1x_sb[:, c * DH1:(c + 1) * DH1],
                    rhs=x_sb[:, sub, c, :],
                    start=(c == 0),
                    stop=(c == KC - 1),
                )

            # h = relu(x@W1 + b1), with the ones-row at partition 100
            h_sb = hp.tile([DH1, TILE], F16)
            nc.scalar.activation(out=h_sb[:, :], in_=hps[:, :],
                                 func=mybir.ActivationFunctionType.Relu,
                                 bias=b1_ap)

            # freq count: (h > 0) summed along the batch (free) dim.
            # h is post-relu fp16; its rounding flips the f32 sign only for
            # |h| < 2^-24, so the count matches the f32 indicator.
            ind_sb = indp.tile([D_HID, TILE], F16)
            nc.vector.tensor_scalar(
                out=ind_sb[:, :],
                in0=h_sb[0:D_HID, :],
                scalar1=0.0,
                scalar2=None,
                op0=mybir.AluOpType.is_gt,
                op1=mybir.AluOpType.add,
                accum_out=freq_sb[:, t:t + 1],
            )
            if t == N_TILES - 1:
                # indicator of the shard's last row (global last on core 7)
                nc.vector.tensor_scalar(
                    out=il_sb[:, :],
                    in0=h_sb[0:D_HID, TILE - 1:TILE],
                    scalar1=0.0,
                    scalar2=None,
                    op0=mybir.AluOpType.is_gt,
                )

            if pending is not None:
                tprev = pending[0]
                softmax_emit(*pending)
                if tprev == N_TILES // 2 - 1:
                    half = N_TILES // 2
                    nc.sync.dma_start(out=out[:, 0:half],
                                      in_=out_all[:, 0:half, :, :])
            pending = (t, h_sb)

    softmax_emit(*pending)

    half = N_TILES // 2
    nc.sync.dma_start(out=out[:, half:], in_=out_all[:, half:, :, :])
    nc.sync.dma_start(out=freqc, in_=freq_sb[:, :])


def build_nc():
    nc = bacc.Bacc("TRN2", target_bir_lowering=False, debug=False,
                   num_devices=N_CORES)
    aps = {
        "xb": nc.dram_tensor("xb", [N_STAGES, KP, ST, KC, TILE], F16,
                             kind="ExternalInput").ap(),
        "W1X": nc.dram_tensor("W1X", [KP, KC * DH1 + D_OUT], F16,
                              kind="ExternalInput").ap(),
        "wc": nc.dram_tensor("wc", [128, 2], F32, kind="ExternalInput").ap(),
        # out is stored blocked [p, t, j, k]; host restores batch order
        "out": nc.dram_tensor("out", [128, N_TILES, J, D_OUT], F32,
                              kind="ExternalOutput").ap(),
        "freqc": nc.dram_tensor("freqc", [D_HID, N_TILES + 1], F32,
                                kind="ExternalOutput").ap(),
    }
    with tile.TileContext(nc) as tc, ExitStack() as ctx:
        _build_body(ctx, tc, aps)
    nc.compile()
    return nc


_NC = None


def _get_nc():
    global _NC
    if _NC is None:
        _NC = build_nc()
    return _NC


def make_in_maps(x, W1, b1, W2, b2):
    xf = np.ascontiguousarray(np.asarray(x, dtype=np.float32).reshape(B, D_IN))
    W1 = np.asarray(W1, dtype=np.float32)
    b1 = np.asarray(b1, dtype=np.float32)
    W2 = np.asarray(W2, dtype=np.float32)
    b2 = np.asarray(b2, dtype=np.float32)
    # host-side weight augmentation + blocking (layout only; see _build_body)
    W1p = np.concatenate([W1, np.zeros((D_IN, 1), np.float32)], axis=1)
    W1X = np.zeros((KP, KC * DH1 + D_OUT), np.float16)
    W1X[:, :KC * DH1] = (W1p.reshape(KC, KP, DH1).transpose(1, 0, 2)
                         .reshape(KP, KC * DH1).astype(np.float16))
    W2b = np.concatenate([W2, b2[None, :]], axis=0)           # [101, 10]
    W1X[0:DH1, KC * DH1:] = W2b.astype(np.float16)
    wc = np.zeros((128, 2), np.float32)
    wc[0:D_HID, 0] = b1
    wc[D_HID, 0] = 1.0
    wc[0:D_HID, 1] = -b1
    in_maps = []
    for c in range(N_CORES):
        shard = xf[c * B_SHARD:(c + 1) * B_SHARD]             # [8192, 784]
        # xb[s, p, sub, ch, col] = shard[(s*ST+sub)*TILE + col, ch*KP + p]
        xbt = shard.reshape(N_STAGES, ST, TILE, KC, KP).transpose(0, 4, 1, 3, 2)
        in_maps.append({
            "xb": np.ascontiguousarray(xbt.astype(np.float16)),
            "W1X": W1X, "wc": wc,
        })
    return in_maps


def postprocess(results, freq):
    outs = []
    for c in range(N_CORES):
        o = results[c]["out"]                             # [128, 16, 4, 10]
        outs.append(o.transpose(1, 2, 0, 3).reshape(B_SHARD, D_OUT))
    out = np.concatenate(outs, axis=0)                    # [65536, 10]
    freq_new = np.asarray(freq, dtype=np.float32).copy()
    for c in range(N_CORES):
        freq_new += results[c]["freqc"][:, :N_TILES].sum(axis=1)
    freq_new -= results[N_CORES - 1]["freqc"][:, N_TILES]
    return out, freq_new.astype(np.float32)


def kernel(x, W1, b1, freq, W2, b2):
    nc = _get_nc()
    in_maps = make_in_maps(x, W1, b1, W2, b2)
    res = run_bass_kernel_spmd(nc, in_maps, list(range(N_CORES)))
    return postprocess(res.results, freq)
